# revision 11
# baseline (speedup 1.0000x reference)
"""Trainium2 Bass kernel for nn_DataEmbedding_cycle_pos.

out[b,t,:] = conv(x) + temporal(x_mark) + cycle-positional, where the
cycle term is a*postab[t] + (cnt/8)*odd with a = 1-cnt/8 and cnt =
#series in batch b whose Nyquist bin is the strict argmax of |rfft|.

HW model (measured): PE mostly ~1.2GHz (HAM warms late), N=512 matmul
~540ns cold / ~385ns warm.  DVE 0.96GHz 1x/2x/4x; ACT 1.2GHz 1x.  DMA
issue ~0.65us on the issuing engine; HWDGE queue ~206GB/s, SWDGE
~141GB/s.  Cross-engine sem hop ~300ns; postamble (sem resets) ~9us.

Structure:
 - lt[b] = [onehot28 | xtp3(24, ones row at 32) ] (53 rows) duplicated
   at partition 64 so two tiles' K=53 matmuls run CONCURRENTLY in
   disjoint PE row groups (~540ns per tile pair).  The ones row at
   index 32 pairs with rhs row 32 = (cnt/8)*odd so rhs rows except one
   are FFT-independent constants.
 - cyc fusion per 128x1024 pair by PAIR_MODE: 'V' = DVE
   scalar_tensor_tensor (1x, reads psum);  'P' = PE aI identity-matmul
   accumulate + ACT psum->sbuf copy.
 - Spectrum: stage-1 K=128 DFT matmul, bf16 twiddle (ACT converts, DVE
   2x TTs), stage-2 4 matmuls per half (negated-Fs table, no negation
   copy).  Strict-max with a FLAT chain: mag=Re^2+Im^2 on DVE, tiny
   selector matmul broadcasts each series' Nyquist value per partition,
   one 4x-mode is_ge tensor_scalar with fused accum_out row-sum counts
   violations, selector matmul sums per series, ==1, ones-matmul
   broadcasts cnt to [128,1].
 - DMA: sync = all input loads + onehot partition-dup (SBUF->SBUF) + 6
   stores; scalar = postab (2x1MB) + 5 stores; gpsimd = 5 stores.
   Output flat [128, BPC*NT*D] so stores are 2KB/partition contiguous.
"""
import sys, os

sys.path.insert(0, "/opt/trn_rl_repo")
import numpy as np
import ml_dtypes

import concourse.bass as bass
import concourse.bacc as bacc
import concourse.mybir as mybir
import concourse.tile as tile
from concourse.bass_utils import run_bass_kernel_spmd

B, T, N, D = 16, 2048, 8, 512
NCORES = 8
BPC = B // NCORES          # batches per core
SPC = BPC * N              # series per core (16)
NT = T // 128              # 128-row time tiles per batch
KCONV = 3 * N              # 24 conv rows
KHOT = 28                  # 4 features x 7 index values
KLT = KHOT + KCONV + 1     # 53 rows: onehot | conv(+ones at idx 32)
NK2 = 9                    # k2 = 0..8 covers bins 0..1151 (Nyquist = (0,8))
NF = NK2 * 8               # 72 stage-2 rows per half

# packed fft-const column offsets:
# [cos128 | -sin128 | twc | tws | Fc | Fs | FsNeg | I | sel72 | sel8]
OC, OS, OTWC, OTWS = 0, 128, 256, 512
OFC, OFS, OFSN, OI = 768, 840, 912, 984
OSEL, OSEL8 = 1112, 1184
CW = OSEL8 + 8

F32 = mybir.dt.float32
BF16 = mybir.dt.bfloat16
BF = ml_dtypes.bfloat16

TRACE = False
TRACE_DIR = None

# per-pair fusion mode, 8 pairs per batch x 2 batches:
#  'V' = DVE scalar_tensor_tensor;  'P' = PE aI-matmuls + ACT copy
PAIR_MODE = ['V', 'P', 'V', 'V', 'P', 'V', 'V', 'P',
             'V', 'P', 'V', 'V', 'P', 'V', 'V', 'V']
# store queue per pair (cycles sync/scalar/gpsimd)
STORE_Q = ['sync', 'scalar', 'gpsimd'] * 6

_cache = {}


# ----------------------------------------------------------------- constants
def _div_term():
    return np.exp(
        np.arange(0, D, 2, dtype=np.float32) * np.float32(-np.log(10000.0) / D)
    ).astype(np.float32)


def _fixed_rows(nrows):
    pos = np.arange(nrows, dtype=np.float32)[:, None]
    ang = (pos * _div_term()[None, :]).astype(np.float32)
    tab = np.zeros((nrows, D), dtype=np.float32)
    tab[:, 0::2] = np.sin(ang)
    tab[:, 1::2] = np.cos(ang)
    return tab


def _host_constants():
    c = {}
    postab = _fixed_rows(T)  # [2048, 512]
    # SBUF layout [128(t%128), 16tiles * 512]
    c["postab"] = np.ascontiguousarray(
        postab.reshape(NT, 128, D).transpose(1, 0, 2).reshape(128, NT * D)
    ).astype(BF)
    r7 = _fixed_rows(7)
    c["r4"] = np.ascontiguousarray(np.tile(r7, (4, 1))).astype(BF)
    c["odd"] = np.zeros((1, D), dtype=np.float32)
    c["odd"][0, 1::2] = 1.0
    c["v28"] = np.tile(np.arange(7, dtype=np.float32), 4)[:, None].copy()

    # stage-1 DFT over t1 (length 128): Y = Yc + i*Ys
    t1 = np.arange(128, dtype=np.float64)
    k1 = np.arange(128, dtype=np.float64)
    a1 = 2.0 * np.pi / 128.0 * np.outer(t1, k1)
    # twiddle e^{-2pi i k1 t2 / T}: [k1, s*16+t2], tiled over the 16 series
    t2 = np.arange(16, dtype=np.float64)
    phi = 2.0 * np.pi / T * np.outer(k1, t2)          # [128, 16]
    # stage-2 tables: rows p = s8*16 + t2, cols c = s8*9 + k2
    fc = np.zeros((128, NF), dtype=np.float64)
    fs = np.zeros((128, NF), dtype=np.float64)
    for s8 in range(8):
        for it2 in range(16):
            p = s8 * 16 + it2
            for k2 in range(NK2):
                ang = 2.0 * np.pi * k2 * it2 / 16.0
                fc[p, s8 * NK2 + k2] = np.cos(ang)
                fs[p, s8 * NK2 + k2] = np.sin(ang)
    # strict-max selectors: sel72[k, p] = 1 iff k = 9*(p//9)+8  (Nyquist
    # row of p's series);  sel8[k, s] = 1 iff k//9 == s
    sel72 = np.zeros((128, 72), dtype=np.float64)
    sel8 = np.zeros((128, 8), dtype=np.float64)
    for p in range(72):
        sel72[9 * (p // 9) + 8, p] = 1.0
    for k in range(72):
        sel8[k, k // 9] = 1.0
    c["fconst"] = np.ascontiguousarray(np.concatenate(
        [np.cos(a1), -np.sin(a1),
         np.tile(np.cos(phi), (1, 16)), np.tile(np.sin(phi), (1, 16)),
         fc, fs, -fs, np.eye(128), sel72, sel8], axis=1)).astype(BF)
    assert c["fconst"].shape == (128, CW)
    return c


# ------------------------------------------------------------------- program
def _build_nc():
    MUL = mybir.AluOpType.mult
    ADD = mybir.AluOpType.add
    nc = bacc.Bacc("TRN2", target_bir_lowering=False, debug=False,
                   num_devices=NCORES)

    def din(name, shape, dt):
        return nc.dram_tensor(name, shape, dt, kind="ExternalInput").ap()

    # ltin rows: [xm7(28) | xtp3-with-ones(25)]
    ltin = din("ltin", [BPC, KLT, T], BF16)
    xrs = din("xrs", [128, SPC * 16], BF16)       # x[16*t1+t2] as [t1, s*16+t2]
    fconst = din("fconst", [128, CW], BF16)
    postab = din("postab", [128, NT * D], BF16)
    w24o = din("w24o", [KCONV + 1, D], BF16)      # [w24[0:4]; odd; w24[4:]]
    r4 = din("r4", [KHOT, D], BF16)
    v28 = din("v28", [KHOT, 1], F32)
    # flat output: out[p, ((i*NT)+ti)*D + d] = full[i, ti*128+p, d]
    out = nc.dram_tensor("out", [128, BPC * NT * D], BF16,
                         kind="ExternalOutput").ap()

    with tile.TileContext(nc) as tc:
        with (
            tc.tile_pool(name="consts", bufs=1) as cpool,
            tc.tile_pool(name="fwork", bufs=1) as fpool,
            tc.tile_pool(name="batch", bufs=2) as bpool,
            tc.tile_pool(name="outp", bufs=1) as opool,
        ):
            lts, rhss = [], []
            for i in range(BPC):
                lts.append(bpool.tile([64 + KLT, T], BF16, tag="lt",
                                      name="lt"))
                rhss.append(bpool.tile([64 + KLT, D], BF16, tag="rhs",
                                       name="rhs"))

            # -------- input DMAs
            # sync (HWDGE, <=8 outstanding): only the 4 critical loads
            fc_sb = cpool.tile([128, CW], BF16, tag="fconst")
            nc.sync.dma_start(fc_sb[:], fconst)
            xrs_sb = fpool.tile([128, SPC * 16], BF16, tag="xrs")
            nc.sync.dma_start(xrs_sb[:], xrs)
            for i in range(BPC):
                nc.sync.dma_start(lts[i][0:KLT, :], ltin[i])
            # gpsimd (SWDGE, fire-and-forget issues): small consts
            v28_sb = cpool.tile([KHOT, 1], F32, tag="v28")
            nc.gpsimd.dma_start(v28_sb[:], v28)
            for i in range(BPC):
                nc.gpsimd.dma_start(rhss[i][0:KHOT, :], r4)
                nc.gpsimd.dma_start(rhss[i][64:64 + KHOT, :], r4)
                nc.gpsimd.dma_start(rhss[i][KHOT:KLT, :], w24o)
                nc.gpsimd.dma_start(rhss[i][64 + KHOT:64 + KLT, :], w24o)
            # scalar: postab (2x1MB) + the B-side xtp3 rows
            ptall = cpool.tile([128, NT, D], BF16, tag="ptall")
            nc.scalar.dma_start(ptall[:, 0:8, :], postab[:, 0:8 * D])
            nc.scalar.dma_start(ptall[:, 8:16, :], postab[:, 8 * D:16 * D])
            for i in range(BPC):
                nc.scalar.dma_start(lts[i][64 + KHOT:64 + KLT, :],
                                    ltin[i, KHOT:KLT])

            # ---------------- FFT phase
            a_vecs, c8_vecs, ai_sbs = [], [], []
            with tc.tile_pool(name="fftps", bufs=1, space="PSUM") as fps:
                ps_yc = fps.tile([128, SPC * 16], F32, tag="yc")
                ps_ys = fps.tile([128, SPC * 16], F32, tag="ys")
                nc.tensor.matmul(ps_yc[:], fc_sb[:, OC:OC + 128], xrs_sb[:],
                                 start=True, stop=True)
                nc.tensor.matmul(ps_ys[:], fc_sb[:, OS:OS + 128], xrs_sb[:],
                                 start=True, stop=True)

                # twiddle: convert stage-1 to bf16 on ACT, then 2x-mode TTs
                ycb = fpool.tile([128, SPC * 16], BF16, tag="ycb")
                ysb = fpool.tile([128, SPC * 16], BF16, tag="ysb")
                nc.scalar.copy(ycb[:], ps_yc[:])
                nc.scalar.copy(ysb[:], ps_ys[:])
                twc, tws = fc_sb[:, OTWC:OTWC + 256], fc_sb[:, OTWS:OTWS + 256]
                tca = fpool.tile([128, SPC * 16], BF16, tag="tca")
                tcb = fpool.tile([128, SPC * 16], BF16, tag="tcb")
                ycp = fpool.tile([128, SPC * 16], BF16, tag="ycp")
                nc.vector.tensor_tensor(tca[:], ycb[:], twc, op=MUL)
                nc.vector.tensor_tensor(tcb[:], ysb[:], tws, op=MUL)
                nc.vector.tensor_add(ycp[:], tca[:], tcb[:])
                tsa = fpool.tile([128, SPC * 16], BF16, tag="tsa")
                tsb = fpool.tile([128, SPC * 16], BF16, tag="tsb")
                ysp = fpool.tile([128, SPC * 16], BF16, tag="ysp")
                nc.vector.tensor_tensor(tsa[:], ysb[:], twc, op=MUL)
                nc.vector.tensor_tensor(tsb[:], ycb[:], tws, op=MUL)
                nc.vector.tensor_sub(ysp[:], tsa[:], tsb[:])

                # onehot in place over the loaded xm rows (A side); DVE
                # is idle here and ltin has landed -- emitting any later
                # risks stalling the strict-FIFO DVE queue
                for h in range(BPC):
                    nc.vector.tensor_scalar(lts[h][0:KHOT, :],
                                            lts[h][0:KHOT, :],
                                            v28_sb[:], None,
                                            op0=mybir.AluOpType.is_equal)
                ones8 = fpool.tile([8, 128], BF16, tag="ones8")
                nc.vector.memset(ones8[:], 1.0)
                i128 = fc_sb[:, OI:OI + 128]
                for h in range(BPC):
                    cs = slice(h * 128, (h + 1) * 128)
                    tp_c = fps.tile([128, 128], BF16, tag="tpc", bufs=1)
                    nc.tensor.transpose(tp_c[:], ycp[:, cs], i128)
                    tp_s = fps.tile([128, 128], BF16, tag="tps", bufs=1)
                    nc.tensor.transpose(tp_s[:], ysp[:, cs], i128)
                    yct = fpool.tile([128, 128], BF16, tag="yct", bufs=2)
                    yst = fpool.tile([128, 128], BF16, tag="yst", bufs=2)
                    nc.scalar.copy(yct[:], tp_c[:])
                    nc.scalar.copy(yst[:], tp_s[:])
                    # stage-2: Re = Fc@yct - Fs@yst, Im = Fs@yct + Fc@yst
                    ps_ri = fps.tile([NF, 256], F32, tag="ri", bufs=1)
                    nc.tensor.matmul(ps_ri[:, 0:128], fc_sb[:, OFC:OFC + NF],
                                     yct[:], start=True, stop=False)
                    nc.tensor.matmul(ps_ri[:, 0:128],
                                     fc_sb[:, OFSN:OFSN + NF],
                                     yst[:], start=False, stop=True)
                    nc.tensor.matmul(ps_ri[:, 128:256],
                                     fc_sb[:, OFS:OFS + NF],
                                     yct[:], start=True, stop=False)
                    nc.tensor.matmul(ps_ri[:, 128:256],
                                     fc_sb[:, OFC:OFC + NF],
                                     yst[:], start=False, stop=True)

                    # mag = Re^2 + Im^2 (ACT squares, DVE add, bf16 SBUF)
                    m1 = fpool.tile([NF, 128], F32, tag="m1", bufs=2)
                    m2 = fpool.tile([NF, 128], F32, tag="m2", bufs=2)
                    mag = fpool.tile([NF, 128], BF16, tag="mag", bufs=2)
                    nc.scalar.square(m1[:], ps_ri[:, 0:128])
                    nc.scalar.square(m2[:], ps_ri[:, 128:256])
                    nc.vector.tensor_add(mag[:], m1[:], m2[:])

                    # nyqb[p] = mag[nyqrow(series(p)), 0] via selector matmul
                    ps_nyqb = fps.tile([NF, 1], F32, tag="nyqb", bufs=1)
                    nc.tensor.matmul(ps_nyqb[:],
                                     fc_sb[0:NF, OSEL:OSEL + NF],
                                     mag[:, 0:1], start=True, stop=True)
                    nyqb = fpool.tile([NF, 1], F32, tag="nyqbs", bufs=2)
                    nc.vector.tensor_copy(nyqb[:], ps_nyqb[:])
                    # count cells >= nyq per row, fused row-sum
                    ge = fpool.tile([NF, 128], BF16, tag="ge", bufs=2)
                    vio = fpool.tile([NF, 1], F32, tag="vio", bufs=2)
                    nc.vector.tensor_scalar(
                        ge[:], mag[:], nyqb[:], None,
                        op0=mybir.AluOpType.is_ge, op1=ADD,
                        accum_out=vio[:])
                    viob = fpool.tile([NF, 1], BF16, tag="viob", bufs=2)
                    nc.vector.tensor_copy(viob[:], vio[:])
                    # per-series violation totals; strict max <=> total == 1
                    ps_vio8 = fps.tile([8, 1], F32, tag="vio8", bufs=1)
                    nc.tensor.matmul(ps_vio8[:],
                                     fc_sb[0:NF, OSEL8:OSEL8 + 8],
                                     viob[:], start=True, stop=True)
                    is1 = fpool.tile([8, 1], BF16, tag="is1", bufs=2)
                    nc.vector.tensor_scalar(is1[:], ps_vio8[:], 1.0, None,
                                            op0=mybir.AluOpType.is_equal)
                    ps_cnt = fps.tile([128, 1], F32, tag="cnt", bufs=1)
                    nc.tensor.matmul(ps_cnt[:], ones8[:], is1[:],
                                     start=True, stop=True)

                    a_vec = fpool.tile([128, 1], F32, tag="avec", bufs=2)
                    nc.vector.tensor_scalar(a_vec[:], ps_cnt[:], -0.125, 1.0,
                                            op0=MUL, op1=ADD)
                    c8_vec = fpool.tile([128, 1], F32, tag="c8vec", bufs=2)
                    nc.vector.tensor_scalar(c8_vec[:], ps_cnt[:], 0.125,
                                            None, op0=MUL)
                    a_vecs.append(a_vec)
                    c8_vecs.append(c8_vec)

                    # stationary for PE-fused pairs: aI = a * I128 (bf16)
                    ai = fpool.tile([128, 128], BF16, tag=f"ai{h}")
                    nc.vector.tensor_scalar(ai[:], i128, a_vecs[h][:], None,
                                            op0=MUL)
                    ai_sbs.append(ai)

                    # rhs odd-row (32-aligned at rows 32 / 96):
                    # (cnt/8) * odd, in place over the DMA-loaded row
                    nc.vector.tensor_scalar(
                        rhss[h][32:33, :], rhss[h][32:33, :],
                        c8_vecs[h][32:33, :], None, op0=MUL)
                    nc.vector.tensor_scalar(
                        rhss[h][96:97, :], rhss[h][96:97, :],
                        c8_vecs[h][96:97, :], None, op0=MUL)

            # onehot partition-64 duplicate via SBUF->SBUF DMA (sync/HWDGE)
            for i in range(BPC):
                nc.sync.dma_start(lts[i][64:64 + KHOT, :], lts[i][0:KHOT, :])

            # ---------------- main per-batch pipelines
            # pack = 2 concurrent K=53 matmuls (rows 0-52 / 64-116) filling
            # one psum pair; fusion per pair by PAIR_MODE.
            with tc.tile_pool(name="mps", bufs=1, space="PSUM") as mps:
                npair = 0
                for i in range(BPC):
                    for pq in range(8):          # pair index within batch
                        mode = PAIR_MODE[npair]
                        stq = STORE_Q[npair]
                        npair += 1
                        psp = mps.tile([128, 2, D], F32, tag="psp", bufs=4)
                        otp = opool.tile([128, 2, D], BF16, tag="otp",
                                         bufs=6)
                        t0, t1b = pq * 2, pq * 2 + 1
                        last = (mode != 'P')
                        nc.tensor.matmul(
                            psp[:, 0, :],
                            lts[i][0:KLT, t0 * 128:(t0 + 1) * 128],
                            rhss[i][0:KLT, :], start=True, stop=last)
                        nc.tensor.matmul(
                            psp[:, 1, :],
                            lts[i][64:64 + KLT, t1b * 128:(t1b + 1) * 128],
                            rhss[i][64:64 + KLT, :], start=True, stop=last)
                        ptpair = ptall[:, 2 * pq:2 * pq + 2, :]
                        if mode == 'P':
                            nc.tensor.matmul(
                                psp[:, 0, :], ai_sbs[i][:],
                                ptall[:, 2 * pq, :],
                                start=False, stop=True)
                            nc.tensor.matmul(
                                psp[:, 1, :], ai_sbs[i][:],
                                ptall[:, 2 * pq + 1, :],
                                start=False, stop=True)
                            nc.scalar.copy(otp[:], psp[:])
                        else:
                            nc.vector.scalar_tensor_tensor(
                                otp[:], ptpair, a_vecs[i][:], psp[:],
                                op0=MUL, op1=ADD)
                        col = (i * NT + pq * 2) * D
                        getattr(nc, stq).dma_start(
                            out[:, col:col + 2 * D], otp[:])
    nc.compile()
    return nc


def _get_nc():
    if "nc" not in _cache:
        _cache["nc"] = _build_nc()
    return _cache["nc"]


def _host_inputs(x, x_mark, conv_w):
    # conv rows (k, n): xtp3[b, 8k+n, t] = x[b, (t-1+k) % T, n]; the ones
    # row sits at conv index 4 so it lands on lt row 32
    xt = x.transpose(0, 2, 1)                              # [16, 8, 2048]
    xtp3 = np.stack([np.roll(xt, 1, axis=2), xt, np.roll(xt, -1, axis=2)],
                    axis=1).reshape(B, KCONV, T)
    ones = np.ones((B, 1, T), dtype=np.float32)
    # x_mark, transposed, each feature row repeated 7x -> [16, 28, T]
    xmt = x_mark.astype(np.float32).transpose(0, 2, 1)     # [16, 4, 2048]
    xm7 = np.repeat(xmt, 7, axis=1)                        # [16, 28, 2048]
    ltin = np.concatenate(
        [xm7, xtp3[:, 0:4], ones, xtp3[:, 4:]], axis=1).astype(BF)
    # conv weight rows (k, n): w24[k*8+n, d] = conv_w[d, n, k]; the odd
    # row sits at index 4, matching the ones row position
    w24f = conv_w.transpose(2, 1, 0).reshape(KCONV, D).astype(np.float32)
    w24o = np.ascontiguousarray(np.concatenate(
        [w24f[0:4], _cache["consts"]["odd"], w24f[4:]], axis=0)).astype(BF)
    # per-core stage-1 operand: xrs[t1, s*16+t2] = x[b0+s//8, 16*t1+t2, s%8]
    xrs_cores = []
    for core in range(NCORES):
        xs = x[core * BPC:(core + 1) * BPC]                # [2, 2048, 8]
        xr = xs.reshape(BPC, 128, 16, N).transpose(1, 0, 3, 2)  # [t1,b,n,t2]
        xrs_cores.append(np.ascontiguousarray(
            xr.reshape(128, SPC * 16)).astype(BF))
    return ltin, w24o, xrs_cores


def make_in_maps(x, x_mark, conv_w):
    if "consts" not in _cache:
        _cache["consts"] = _host_constants()
    c = _cache["consts"]
    ltin, w24o, xrs_cores = _host_inputs(x, x_mark, conv_w)
    in_maps = []
    for core in range(NCORES):
        b0 = core * BPC
        in_maps.append({
            "ltin": np.ascontiguousarray(ltin[b0:b0 + BPC]),
            "xrs": xrs_cores[core],
            "fconst": c["fconst"],
            "postab": c["postab"],
            "w24o": w24o,
            "r4": c["r4"],
            "v28": c["v28"],
        })
    return in_maps


def _unshard(buf):
    # buf [128, BPC*NT*D] -> [BPC, T, D]
    a = np.asarray(buf).reshape(128, BPC, NT, D)
    return a.transpose(1, 2, 0, 3).reshape(BPC, T, D)


# -------------------------------------------------------------------- driver
def kernel(**inputs):
    x = np.asarray(inputs["x"], dtype=np.float32)          # [16, 2048, 8]
    x_mark = np.asarray(inputs["x_mark"])                  # [16, 2048, 4] int
    conv_w = np.asarray(inputs["conv_w"], dtype=np.float32)  # [512, 8, 3]

    in_maps = make_in_maps(x, x_mark, conv_w)
    nc = _get_nc()
    kw = {}
    if TRACE:
        kw = dict(trace=True, tmpdir=TRACE_DIR)
    br = run_bass_kernel_spmd(nc, in_maps, list(range(NCORES)), **kw)
    if TRACE:
        _cache["last_results"] = br

    outp = np.empty((B, T, D), dtype=np.float32)
    for core in range(NCORES):
        outp[core * BPC:(core + 1) * BPC] = \
            _unshard(br.results[core]["out"]).astype(np.float32)
    return outp


# revision 13
# speedup vs baseline: 1.6046x; 1.6046x over previous
"""Trainium2 Bass kernel for nn_DataEmbedding_cycle_pos.

out[b,t,:] = conv(x) + temporal(x_mark) + cycle-positional, where the
cycle term is a*postab[t] + (cnt/8)*odd with a = 1-cnt/8 and cnt =
#series in batch b whose Nyquist bin is the strict argmax of |rfft|.

HW model (measured): PE mostly ~1.2GHz (HAM warms late), N=512 matmul
~540ns cold / ~385ns warm.  DVE 0.96GHz 1x/2x/4x; ACT 1.2GHz 1x.  DMA
issue ~0.65us on the issuing engine; HWDGE queue ~206GB/s, SWDGE
~141GB/s.  Cross-engine sem hop ~300ns; postamble (sem resets) ~9us.

Structure:
 - lt[b] = [onehot28 | xtp3(24, ones row at 32) ] (53 rows) duplicated
   at partition 64 so two tiles' K=53 matmuls run CONCURRENTLY in
   disjoint PE row groups (~540ns per tile pair).  The ones row at
   index 32 pairs with rhs row 32 = (cnt/8)*odd so rhs rows except one
   are FFT-independent constants.
 - cyc fusion per 128x1024 pair by PAIR_MODE: 'V' = DVE
   scalar_tensor_tensor (1x, reads psum);  'P' = PE aI identity-matmul
   accumulate + ACT psum->sbuf copy.
 - Spectrum: stage-1 K=128 DFT matmul, bf16 twiddle (ACT converts, DVE
   2x TTs), stage-2 4 matmuls per half (negated-Fs table, no negation
   copy).  Strict-max with a FLAT chain: mag=Re^2+Im^2 on DVE, tiny
   selector matmul broadcasts each series' Nyquist value per partition,
   one 4x-mode is_ge tensor_scalar with fused accum_out row-sum counts
   violations, selector matmul sums per series, ==1, ones-matmul
   broadcasts cnt to [128,1].
 - DMA: sync = all input loads + onehot partition-dup (SBUF->SBUF) + 6
   stores; scalar = postab (2x1MB) + 5 stores; gpsimd = 5 stores.
   Output flat [128, BPC*NT*D] so stores are 2KB/partition contiguous.
"""
import sys, os

sys.path.insert(0, "/opt/trn_rl_repo")
import numpy as np
import ml_dtypes

import concourse.bass as bass
import concourse.bacc as bacc
import concourse.mybir as mybir
import concourse.tile as tile
from concourse.bass_utils import run_bass_kernel_spmd

B, T, N, D = 16, 2048, 8, 512
NCORES = 8
BPC = B // NCORES          # batches per core
SPC = BPC * N              # series per core (16)
NT = T // 128              # 128-row time tiles per batch
KCONV = 3 * N              # 24 conv rows
KHOT = 28                  # 4 features x 7 index values
KLT = KHOT + KCONV + 1     # 53 rows: onehot | conv(+ones at idx 32)
NK2 = 9                    # k2 = 0..8 covers bins 0..1151 (Nyquist = (0,8))
NF = NK2 * 8               # 72 stage-2 rows per half

# packed fft-const column offsets:
# [cos128 | -sin128 | twc | tws | Fc | Fs | FsNeg | I | sel72 | sel8]
OC, OS, OTWC, OTWS = 0, 128, 256, 512
OFC, OFS, OFSN, OI = 768, 840, 912, 984
OSEL, OSEL8 = 1112, 1184
CW = OSEL8 + 8

F32 = mybir.dt.float32
BF16 = mybir.dt.bfloat16
BF = ml_dtypes.bfloat16

TRACE = False
TRACE_DIR = None

# per-pair fusion mode, 8 pairs per batch x 2 batches:
#  'V' = DVE scalar_tensor_tensor;  'P' = PE aI-matmuls + ACT copy
PAIR_MODE = ['V', 'P', 'V', 'V', 'P', 'V', 'V', 'P',
             'V', 'P', 'V', 'V', 'P', 'V', 'V', 'V']
# store queue per pair (cycles sync/scalar/gpsimd)
STORE_Q = ['sync', 'scalar', 'gpsimd'] * 6

_cache = {}


# ----------------------------------------------------------------- constants
def _div_term():
    return np.exp(
        np.arange(0, D, 2, dtype=np.float32) * np.float32(-np.log(10000.0) / D)
    ).astype(np.float32)


def _fixed_rows(nrows):
    pos = np.arange(nrows, dtype=np.float32)[:, None]
    ang = (pos * _div_term()[None, :]).astype(np.float32)
    tab = np.zeros((nrows, D), dtype=np.float32)
    tab[:, 0::2] = np.sin(ang)
    tab[:, 1::2] = np.cos(ang)
    return tab


def _host_constants():
    c = {}
    postab = _fixed_rows(T)  # [2048, 512]
    # SBUF layout [128(t%128), 16tiles * 512]
    c["postab"] = np.ascontiguousarray(
        postab.reshape(NT, 128, D).transpose(1, 0, 2).reshape(128, NT * D)
    ).astype(BF)
    r7 = _fixed_rows(7)
    c["r4"] = np.ascontiguousarray(np.tile(r7, (4, 1))).astype(BF)
    c["odd"] = np.zeros((1, D), dtype=np.float32)
    c["odd"][0, 1::2] = 1.0
    c["v28"] = np.tile(np.arange(7, dtype=np.float32), 4)[:, None].copy()

    # stage-1 DFT over t1 (length 128): Y = Yc + i*Ys
    t1 = np.arange(128, dtype=np.float64)
    k1 = np.arange(128, dtype=np.float64)
    a1 = 2.0 * np.pi / 128.0 * np.outer(t1, k1)
    # twiddle e^{-2pi i k1 t2 / T}: [k1, s*16+t2], tiled over the 16 series
    t2 = np.arange(16, dtype=np.float64)
    phi = 2.0 * np.pi / T * np.outer(k1, t2)          # [128, 16]
    # stage-2 tables: rows p = s8*16 + t2, cols c = s8*9 + k2
    fc = np.zeros((128, NF), dtype=np.float64)
    fs = np.zeros((128, NF), dtype=np.float64)
    for s8 in range(8):
        for it2 in range(16):
            p = s8 * 16 + it2
            for k2 in range(NK2):
                ang = 2.0 * np.pi * k2 * it2 / 16.0
                fc[p, s8 * NK2 + k2] = np.cos(ang)
                fs[p, s8 * NK2 + k2] = np.sin(ang)
    # strict-max selectors: sel72[k, p] = 1 iff k = 9*(p//9)+8  (Nyquist
    # row of p's series);  sel8[k, s] = 1 iff k//9 == s
    sel72 = np.zeros((128, 72), dtype=np.float64)
    sel8 = np.zeros((128, 8), dtype=np.float64)
    for p in range(72):
        sel72[9 * (p // 9) + 8, p] = 1.0
    for k in range(72):
        sel8[k, k // 9] = 1.0
    c["fconst"] = np.ascontiguousarray(np.concatenate(
        [np.cos(a1), -np.sin(a1),
         np.tile(np.cos(phi), (1, 16)), np.tile(np.sin(phi), (1, 16)),
         fc, fs, -fs, np.eye(128), sel72, sel8], axis=1)).astype(BF)
    assert c["fconst"].shape == (128, CW)
    return c


# ------------------------------------------------------------------- program
def _build_nc():
    MUL = mybir.AluOpType.mult
    ADD = mybir.AluOpType.add
    nc = bacc.Bacc("TRN2", target_bir_lowering=False, debug=False,
                   num_devices=NCORES)

    def din(name, shape, dt):
        return nc.dram_tensor(name, shape, dt, kind="ExternalInput").ap()

    # ltin: full 128-row image, [onehot28|xtp3+ones] at rows 0 and 64
    ltin = din("ltin", [BPC, 128, T], BF16)
    xrs = din("xrs", [128, SPC * 16], BF16)       # x[16*t1+t2] as [t1, s*16+t2]
    fconst = din("fconst", [128, CW], BF16)
    postab = din("postab", [128, NT * D], BF16)
    rhsin = din("rhsin", [128, D], BF16)   # [r4;w24o] at rows 0/64
    # flat output: out[p, ((i*NT)+ti)*D + d] = full[i, ti*128+p, d]
    out = nc.dram_tensor("out", [128, BPC * NT * D], BF16,
                         kind="ExternalOutput").ap()

    with tile.TileContext(nc) as tc:
        with (
            tc.tile_pool(name="consts", bufs=1) as cpool,
            tc.tile_pool(name="fwork", bufs=1) as fpool,
            tc.tile_pool(name="batch", bufs=2) as bpool,
            tc.tile_pool(name="outp", bufs=1) as opool,
        ):
            lts, rhss = [], []
            for i in range(BPC):
                lts.append(bpool.tile([128, T], BF16, tag="lt",
                                      name="lt"))
                rhss.append(bpool.tile([128, D], BF16, tag="rhs",
                                       name="rhs"))

            # -------- input DMAs: all 128-partition-shaped (skinny
            # DMAs measured pathologically slow on this system)
            fc_sb = cpool.tile([128, CW], BF16, tag="fconst")
            nc.sync.dma_start(fc_sb[:], fconst)
            xrs_sb = fpool.tile([128, SPC * 16], BF16, tag="xrs")
            nc.sync.dma_start(xrs_sb[:], xrs)
            for i in range(BPC):
                nc.sync.dma_start(lts[i][:], ltin[i])
            # scalar: rhs images then postab (2x1MB)
            for i in range(BPC):
                nc.scalar.dma_start(rhss[i][:], rhsin)
            ptall = cpool.tile([128, NT, D], BF16, tag="ptall")
            nc.scalar.dma_start(ptall[:, 0:8, :], postab[:, 0:8 * D])
            nc.scalar.dma_start(ptall[:, 8:16, :], postab[:, 8 * D:16 * D])

            # ---------------- FFT phase
            a_vecs, c8_vecs, ai_sbs = [], [], []
            with tc.tile_pool(name="fftps", bufs=1, space="PSUM") as fps:
                ps_yc = fps.tile([128, SPC * 16], F32, tag="yc")
                ps_ys = fps.tile([128, SPC * 16], F32, tag="ys")
                nc.tensor.matmul(ps_yc[:], fc_sb[:, OC:OC + 128], xrs_sb[:],
                                 start=True, stop=True)
                nc.tensor.matmul(ps_ys[:], fc_sb[:, OS:OS + 128], xrs_sb[:],
                                 start=True, stop=True)

                # twiddle: convert stage-1 to bf16 on ACT, then 2x-mode TTs
                ycb = fpool.tile([128, SPC * 16], BF16, tag="ycb")
                ysb = fpool.tile([128, SPC * 16], BF16, tag="ysb")
                nc.scalar.copy(ycb[:], ps_yc[:])
                nc.scalar.copy(ysb[:], ps_ys[:])
                twc, tws = fc_sb[:, OTWC:OTWC + 256], fc_sb[:, OTWS:OTWS + 256]
                tca = fpool.tile([128, SPC * 16], BF16, tag="tca")
                tcb = fpool.tile([128, SPC * 16], BF16, tag="tcb")
                ycp = fpool.tile([128, SPC * 16], BF16, tag="ycp")
                nc.vector.tensor_tensor(tca[:], ycb[:], twc, op=MUL)
                nc.vector.tensor_tensor(tcb[:], ysb[:], tws, op=MUL)
                nc.vector.tensor_add(ycp[:], tca[:], tcb[:])
                tsa = fpool.tile([128, SPC * 16], BF16, tag="tsa")
                tsb = fpool.tile([128, SPC * 16], BF16, tag="tsb")
                ysp = fpool.tile([128, SPC * 16], BF16, tag="ysp")
                nc.vector.tensor_tensor(tsa[:], ysb[:], twc, op=MUL)
                nc.vector.tensor_tensor(tsb[:], ycb[:], tws, op=MUL)
                nc.vector.tensor_sub(ysp[:], tsa[:], tsb[:])

                ones8 = fpool.tile([8, 128], BF16, tag="ones8")
                nc.vector.memset(ones8[:], 1.0)
                i128 = fc_sb[:, OI:OI + 128]
                for h in range(BPC):
                    cs = slice(h * 128, (h + 1) * 128)
                    tp_c = fps.tile([128, 128], BF16, tag="tpc", bufs=1)
                    nc.tensor.transpose(tp_c[:], ycp[:, cs], i128)
                    tp_s = fps.tile([128, 128], BF16, tag="tps", bufs=1)
                    nc.tensor.transpose(tp_s[:], ysp[:, cs], i128)
                    yct = fpool.tile([128, 128], BF16, tag="yct", bufs=2)
                    yst = fpool.tile([128, 128], BF16, tag="yst", bufs=2)
                    nc.scalar.copy(yct[:], tp_c[:])
                    nc.scalar.copy(yst[:], tp_s[:])
                    # stage-2: Re = Fc@yct - Fs@yst, Im = Fs@yct + Fc@yst
                    ps_ri = fps.tile([NF, 256], F32, tag="ri", bufs=1)
                    nc.tensor.matmul(ps_ri[:, 0:128], fc_sb[:, OFC:OFC + NF],
                                     yct[:], start=True, stop=False)
                    nc.tensor.matmul(ps_ri[:, 0:128],
                                     fc_sb[:, OFSN:OFSN + NF],
                                     yst[:], start=False, stop=True)
                    nc.tensor.matmul(ps_ri[:, 128:256],
                                     fc_sb[:, OFS:OFS + NF],
                                     yct[:], start=True, stop=False)
                    nc.tensor.matmul(ps_ri[:, 128:256],
                                     fc_sb[:, OFC:OFC + NF],
                                     yst[:], start=False, stop=True)

                    # mag = Re^2 + Im^2 (ACT squares, DVE add, bf16 SBUF)
                    m1 = fpool.tile([NF, 128], F32, tag="m1", bufs=2)
                    m2 = fpool.tile([NF, 128], F32, tag="m2", bufs=2)
                    mag = fpool.tile([NF, 128], BF16, tag="mag", bufs=2)
                    nc.scalar.square(m1[:], ps_ri[:, 0:128])
                    nc.scalar.square(m2[:], ps_ri[:, 128:256])
                    nc.vector.tensor_add(mag[:], m1[:], m2[:])

                    # nyqb[p] = mag[nyqrow(series(p)), 0] via selector matmul
                    ps_nyqb = fps.tile([NF, 1], F32, tag="nyqb", bufs=1)
                    nc.tensor.matmul(ps_nyqb[:],
                                     fc_sb[0:NF, OSEL:OSEL + NF],
                                     mag[:, 0:1], start=True, stop=True)
                    nyqb = fpool.tile([NF, 1], F32, tag="nyqbs", bufs=2)
                    nc.vector.tensor_copy(nyqb[:], ps_nyqb[:])
                    # count cells >= nyq per row, fused row-sum
                    ge = fpool.tile([NF, 128], BF16, tag="ge", bufs=2)
                    vio = fpool.tile([NF, 1], F32, tag="vio", bufs=2)
                    nc.vector.tensor_scalar(
                        ge[:], mag[:], nyqb[:], None,
                        op0=mybir.AluOpType.is_ge, op1=ADD,
                        accum_out=vio[:])
                    viob = fpool.tile([NF, 1], BF16, tag="viob", bufs=2)
                    nc.vector.tensor_copy(viob[:], vio[:])
                    # per-series violation totals; strict max <=> total == 1
                    ps_vio8 = fps.tile([8, 1], F32, tag="vio8", bufs=1)
                    nc.tensor.matmul(ps_vio8[:],
                                     fc_sb[0:NF, OSEL8:OSEL8 + 8],
                                     viob[:], start=True, stop=True)
                    is1 = fpool.tile([8, 1], BF16, tag="is1", bufs=2)
                    nc.vector.tensor_scalar(is1[:], ps_vio8[:], 1.0, None,
                                            op0=mybir.AluOpType.is_equal)
                    ps_cnt = fps.tile([128, 1], F32, tag="cnt", bufs=1)
                    nc.tensor.matmul(ps_cnt[:], ones8[:], is1[:],
                                     start=True, stop=True)

                    a_vec = fpool.tile([128, 1], F32, tag="avec", bufs=2)
                    nc.vector.tensor_scalar(a_vec[:], ps_cnt[:], -0.125, 1.0,
                                            op0=MUL, op1=ADD)
                    c8_vec = fpool.tile([128, 1], F32, tag="c8vec", bufs=2)
                    nc.vector.tensor_scalar(c8_vec[:], ps_cnt[:], 0.125,
                                            None, op0=MUL)
                    a_vecs.append(a_vec)
                    c8_vecs.append(c8_vec)

                    # stationary for PE-fused pairs: aI = a * I128 (bf16)
                    ai = fpool.tile([128, 128], BF16, tag=f"ai{h}")
                    nc.vector.tensor_scalar(ai[:], i128, a_vecs[h][:], None,
                                            op0=MUL)
                    ai_sbs.append(ai)

                    # rhs odd-row (32-aligned at rows 32 / 96):
                    # (cnt/8) * odd, in place over the DMA-loaded row
                    nc.vector.tensor_scalar(
                        rhss[h][32:33, :], rhss[h][32:33, :],
                        c8_vecs[h][32:33, :], None, op0=MUL)
                    nc.vector.tensor_scalar(
                        rhss[h][96:97, :], rhss[h][96:97, :],
                        c8_vecs[h][96:97, :], None, op0=MUL)

            # ---------------- main per-batch pipelines
            # pack = 2 concurrent K=53 matmuls (rows 0-52 / 64-116) filling
            # one psum pair; fusion per pair by PAIR_MODE.
            with tc.tile_pool(name="mps", bufs=1, space="PSUM") as mps:
                npair = 0
                for i in range(BPC):
                    for pq in range(8):          # pair index within batch
                        mode = PAIR_MODE[npair]
                        stq = STORE_Q[npair]
                        npair += 1
                        psp = mps.tile([128, 2, D], F32, tag="psp", bufs=4)
                        otp = opool.tile([128, 2, D], BF16, tag="otp",
                                         bufs=6)
                        t0, t1b = pq * 2, pq * 2 + 1
                        last = (mode != 'P')
                        nc.tensor.matmul(
                            psp[:, 0, :],
                            lts[i][0:KLT, t0 * 128:(t0 + 1) * 128],
                            rhss[i][0:KLT, :], start=True, stop=last)
                        nc.tensor.matmul(
                            psp[:, 1, :],
                            lts[i][64:64 + KLT, t1b * 128:(t1b + 1) * 128],
                            rhss[i][64:64 + KLT, :], start=True, stop=last)
                        ptpair = ptall[:, 2 * pq:2 * pq + 2, :]
                        if mode == 'P':
                            nc.tensor.matmul(
                                psp[:, 0, :], ai_sbs[i][:],
                                ptall[:, 2 * pq, :],
                                start=False, stop=True)
                            nc.tensor.matmul(
                                psp[:, 1, :], ai_sbs[i][:],
                                ptall[:, 2 * pq + 1, :],
                                start=False, stop=True)
                            nc.scalar.copy(otp[:], psp[:])
                        else:
                            nc.vector.scalar_tensor_tensor(
                                otp[:], ptpair, a_vecs[i][:], psp[:],
                                op0=MUL, op1=ADD)
                        col = (i * NT + pq * 2) * D
                        getattr(nc, stq).dma_start(
                            out[:, col:col + 2 * D], otp[:])
    nc.compile()
    return nc


def _get_nc():
    if "nc" not in _cache:
        _cache["nc"] = _build_nc()
    return _cache["nc"]


def _host_inputs(x, x_mark, conv_w):
    # conv rows (k, n): xtp3[b, 8k+n, t] = x[b, (t-1+k) % T, n]; the ones
    # row sits at conv index 4 so it lands on lt row 32
    xt = x.transpose(0, 2, 1)                              # [16, 8, 2048]
    xtp3 = np.stack([np.roll(xt, 1, axis=2), xt, np.roll(xt, -1, axis=2)],
                    axis=1).reshape(B, KCONV, T)
    ones = np.ones((B, 1, T), dtype=np.float32)
    # onehot rows computed on host: oh[b, 7f+k, t] = (x_mark[b,t,f] == k)
    xmt = x_mark.astype(np.int64).transpose(0, 2, 1)       # [16, 4, 2048]
    oh = (xmt[:, :, None, :] ==
          np.arange(7)[None, None, :, None]).reshape(B, KHOT, T)
    lt1 = np.concatenate(
        [oh.astype(np.float32), xtp3[:, 0:4], ones, xtp3[:, 4:]], axis=1)
    ltin = np.zeros((B, 128, T), dtype=np.float32)
    ltin[:, 0:KLT] = lt1
    ltin[:, 64:64 + KLT] = lt1
    ltin = ltin.astype(BF)                                 # [16, 128, 2048]
    # conv weight rows (k, n): w24[k*8+n, d] = conv_w[d, n, k]; the odd
    # row sits at index 4, matching the ones row position
    w24f = conv_w.transpose(2, 1, 0).reshape(KCONV, D).astype(np.float32)
    w24o = np.concatenate(
        [w24f[0:4], _cache["consts"]["odd"], w24f[4:]], axis=0)
    half = np.concatenate(
        [_cache["consts"]["r4"].astype(np.float32), w24o], axis=0)  # [53, D]
    rhsin = np.zeros((128, D), dtype=np.float32)
    rhsin[0:KLT] = half
    rhsin[64:64 + KLT] = half
    rhsin = np.ascontiguousarray(rhsin).astype(BF)
    # per-core stage-1 operand: xrs[t1, s*16+t2] = x[b0+s//8, 16*t1+t2, s%8]
    xrs_cores = []
    for core in range(NCORES):
        xs = x[core * BPC:(core + 1) * BPC]                # [2, 2048, 8]
        xr = xs.reshape(BPC, 128, 16, N).transpose(1, 0, 3, 2)  # [t1,b,n,t2]
        xrs_cores.append(np.ascontiguousarray(
            xr.reshape(128, SPC * 16)).astype(BF))
    return ltin, rhsin, xrs_cores


def make_in_maps(x, x_mark, conv_w):
    if "consts" not in _cache:
        _cache["consts"] = _host_constants()
    c = _cache["consts"]
    ltin, rhsin, xrs_cores = _host_inputs(x, x_mark, conv_w)
    in_maps = []
    for core in range(NCORES):
        b0 = core * BPC
        in_maps.append({
            "ltin": np.ascontiguousarray(ltin[b0:b0 + BPC]),
            "xrs": xrs_cores[core],
            "fconst": c["fconst"],
            "postab": c["postab"],
            "rhsin": rhsin,
        })
    return in_maps


def _unshard(buf):
    # buf [128, BPC*NT*D] -> [BPC, T, D]
    a = np.asarray(buf).reshape(128, BPC, NT, D)
    return a.transpose(1, 2, 0, 3).reshape(BPC, T, D)


# -------------------------------------------------------------------- driver
def kernel(**inputs):
    x = np.asarray(inputs["x"], dtype=np.float32)          # [16, 2048, 8]
    x_mark = np.asarray(inputs["x_mark"])                  # [16, 2048, 4] int
    conv_w = np.asarray(inputs["conv_w"], dtype=np.float32)  # [512, 8, 3]

    in_maps = make_in_maps(x, x_mark, conv_w)
    nc = _get_nc()
    kw = {}
    if TRACE:
        kw = dict(trace=True, tmpdir=TRACE_DIR)
    br = run_bass_kernel_spmd(nc, in_maps, list(range(NCORES)), **kw)
    if TRACE:
        _cache["last_results"] = br

    outp = np.empty((B, T, D), dtype=np.float32)
    for core in range(NCORES):
        outp[core * BPC:(core + 1) * BPC] = \
            _unshard(br.results[core]["out"]).astype(np.float32)
    return outp
